# revision 17
# baseline (speedup 1.0000x reference)
"""Additive (Bahdanau-style) attention kernel for Trainium2, 8 NeuronCores.

Reference computation (per problem):
    hid_att = hid @ W_hid + b_hid                        # (B, A)
    emb_att = einsum('sbe,ea->sba', emb, W_emb) + b_emb  # (S, B, A)
    att     = tanh(emb_att + hid_att[None])              # (S, B, A)
    scores  = einsum('sba,a->bs', att, att_v)            # (B, S)
    weights = softmax(scores, axis=-1)                   # (B, S)
    context = einsum('bs,sbe->be', weights, emb)         # (B, E)
    returns (context, weights)

Sharding: data-parallel over batch. Each of the 8 cores gets 4 batch rows;
projection weights and att_v are replicated. emb is shipped per-core in
(b, e, s) layout so that the E-contraction matmul needs no on-chip
transposes (E lands on the SBUF partition axis naturally), and the
S-contraction (context accumulation) runs on the vector engine as
per-partition dot products over the free (s) axis.

Device dataflow per core (B_LOC=4 local batches, S=2048, E=1024, A=512):
  for b in 4, for chunk c in 4 (512 s each):
    DMA embT[b, :, c] -> SBUF (128 part = e_in, 8 eblk, 512 s)
    PE:  att_ps[ab] (128 a, 512 s) = sum_eb W_emb[eb,ab].T @ embT[eb]   (f32r)
    ACT: att_sb = tanh(att_ps + (hid_att[a,b] + b_emb[a] + b_hid[a]))
    PE:  scores_ps (4, 512 s) = sum_ab v[ab].T(replicated) @ att_sb[ab]
    ACT: w = exp(scores) -> wts_sb row 0 slice   (scores bounded ~|61| < 88)
    GPS: partition_broadcast w -> (128, 512)
    DVE: ctx_acc[b][:, eb, c] = sum_s embT[eb] * w   (tensor_tensor_reduce)
  per-b tail: denom = sum(w row), weights = w/denom, ctx = ctx_acc/denom,
  PE-transpose ctx (128,8)->(8,128), DMA out.
"""

import os
import sys

import numpy as np

if "/opt/trn_rl_repo" not in sys.path:
    sys.path.insert(0, "/opt/trn_rl_repo")

ATT_DIM, HID_DIM, EMB_DIM = 512, 1024, 1024
SEQ_LEN, BATCH = 2048, 32
N_CORES = 8
B_LOC = BATCH // N_CORES          # 4 batch rows per core
EB = EMB_DIM // 128               # 8 e-blocks
AB = ATT_DIM // 128               # 4 a-blocks
CS = 512                          # s chunk size
NCH = SEQ_LEN // CS               # 4 chunks per batch row

# Tensor-engine matmul dtype: "f32r" (relaxed fp32, 4x faster) or "f32".
MM_DT = os.environ.get("KERNEL_MM_DT", "f32r")
# Debug: 1=matmul+tanh+scores+exp+wts, 2=+pass2/ctx, 3=full (normalize)
STAGE = int(os.environ.get("KERNEL_STAGE", "3"))
# eblk index below which pass-2 muls go to DVE (rest to gpsimd)
GPS_SPLIT = int(os.environ.get("KERNEL_GPS_SPLIT", "6"))
# pass-2 implementation: "ttr" (fused mul+reduce, qr.py-style dummy out)
# or "split" (tensor_mul on DVE/gpsimd + wide DVE reduce)
PASS2 = os.environ.get("KERNEL_PASS2", "split")
EMB_BUFS = int(os.environ.get("KERNEL_EMB_BUFS", "5"))

_CACHE = {}


def _build(nc_mod):
    import concourse.bass as bass
    import concourse.tile as tile
    from concourse import bacc, mybir
    from concourse.masks import make_identity

    f32 = mybir.dt.float32
    mm_dt = mybir.dt.float32r if MM_DT == "f32r" else mybir.dt.float32

    nc = bacc.Bacc("TRN2", target_bir_lowering=False, debug=False)

    embT = nc.dram_tensor("embT", [B_LOC, EMB_DIM, SEQ_LEN], f32, kind="ExternalInput").ap()
    hid = nc.dram_tensor("hid", [B_LOC, HID_DIM], f32, kind="ExternalInput").ap()
    W_emb = nc.dram_tensor("W_emb", [EMB_DIM, ATT_DIM], f32, kind="ExternalInput").ap()
    W_hid = nc.dram_tensor("W_hid", [HID_DIM, ATT_DIM], f32, kind="ExternalInput").ap()
    b_emb = nc.dram_tensor("b_emb", [ATT_DIM], f32, kind="ExternalInput").ap()
    b_hid = nc.dram_tensor("b_hid", [ATT_DIM], f32, kind="ExternalInput").ap()
    att_v = nc.dram_tensor("att_v", [ATT_DIM], f32, kind="ExternalInput").ap()
    ctx_out = nc.dram_tensor("ctx", [B_LOC, EMB_DIM], f32, kind="ExternalOutput").ap()
    wts_out = nc.dram_tensor("wts", [B_LOC, SEQ_LEN], f32, kind="ExternalOutput").ap()

    from contextlib import ExitStack

    with tile.TileContext(nc) as tc, ExitStack() as ctx:
        consts = ctx.enter_context(tc.tile_pool(name="consts", bufs=1))
        embp = ctx.enter_context(tc.tile_pool(name="embp", bufs=EMB_BUFS))
        attp = ctx.enter_context(tc.tile_pool(name="attp", bufs=3))
        wbcp = ctx.enter_context(tc.tile_pool(name="wbcp", bufs=2))
        scrp = ctx.enter_context(tc.tile_pool(name="scrp", bufs=2))
        ctxp = ctx.enter_context(tc.tile_pool(name="ctxp", bufs=2))
        ps_att = ctx.enter_context(tc.tile_pool(name="ps_att", bufs=6, space="PSUM"))
        ps_misc = ctx.enter_context(tc.tile_pool(name="ps_misc", bufs=2, space="PSUM"))

        # ---------------- constants / init ----------------
        ident = consts.tile([128, 128], f32)
        make_identity(nc, ident)

        W_emb_sb = consts.tile([128, EB, ATT_DIM], mm_dt)
        nc.sync.dma_start(
            out=W_emb_sb,
            in_=W_emb.rearrange("(eb p) a -> p eb a", p=128).bitcast(mm_dt),
        )
        # W_hid is only needed during init: borrow an embT pool slot
        W_hid_sb = embp.tile([128, EB, ATT_DIM], f32, tag="embT_t")
        nc.sync.dma_start(
            out=W_hid_sb, in_=W_hid.rearrange("(eb p) a -> p eb a", p=128)
        )

        hid_sb = consts.tile([B_LOC, HID_DIM], f32)
        nc.sync.dma_start(out=hid_sb, in_=hid)

        bh_sb = consts.tile([128, AB], f32)
        nc.sync.dma_start(out=bh_sb, in_=b_hid.rearrange("(ab p) -> p ab", p=128))
        be_sb = consts.tile([128, AB], f32)
        nc.sync.dma_start(out=be_sb, in_=b_emb.rearrange("(ab p) -> p ab", p=128))
        bsum_sb = consts.tile([128, AB], f32)
        nc.vector.tensor_add(bsum_sb, bh_sb, be_sb)

        # v replicated to 128 stationary cols so the scores matmul emits all
        # 128 psum rows = scores (a free partition-broadcast for pass 2).
        # Replication via K=1 matmul: out[m,n] = v_row[0,m] * ones[0,n].
        v_row = consts.tile([1, ATT_DIM], f32)
        nc.sync.dma_start(out=v_row, in_=att_v.unsqueeze(0))
        ones_row = consts.tile([1, 128], f32)
        nc.vector.memset(ones_row, 1.0)
        v_sb = consts.tile([128, AB, 128], mm_dt)
        for ab in range(AB):
            v_ps = ps_misc.tile([128, 128], f32, tag="m")
            nc.tensor.matmul(
                v_ps, v_row[0:1, ab * 128 : (ab + 1) * 128], ones_row,
                start=True, stop=True,
            )
            nc.scalar.activation(
                out=v_sb[:, ab, :], in_=v_ps,
                func=mybir.ActivationFunctionType.Copy,
            )

        # hidT: (B_LOC, E) -> (128 e_in, EB, B_LOC) via PE transposes
        hidT_sb = consts.tile([128, EB, B_LOC], f32)
        for eb in range(EB):
            tp_ps = ps_misc.tile([128, B_LOC], f32, tag="m")
            nc.tensor.transpose(
                tp_ps, hid_sb[:, eb * 128 : (eb + 1) * 128], ident[0:B_LOC, 0:B_LOC]
            )
            nc.scalar.activation(
                out=hidT_sb[:, eb, :], in_=tp_ps,
                func=mybir.ActivationFunctionType.Copy,
            )

        # bias_sb[p, ab, b] = hid_att[b, ab*128+p] + b_emb[.] + b_hid[.]
        bias_sb = consts.tile([128, AB, B_LOC], f32)
        for ab in range(AB):
            hat_ps = ps_misc.tile([128, B_LOC], f32, tag="m")
            for eb in range(EB):
                nc.tensor.matmul(
                    hat_ps,
                    W_hid_sb[:, eb, ab * 128 : (ab + 1) * 128],
                    hidT_sb[:, eb, :],
                    start=(eb == 0),
                    stop=(eb == EB - 1),
                )
            nc.scalar.activation(
                out=bias_sb[:, ab, :], in_=hat_ps,
                func=mybir.ActivationFunctionType.Identity,
                bias=bsum_sb[:, ab : ab + 1],
            )

        # weights staging: row b on partition 32*b (engine ops need
        # 32-aligned start partitions; w_bc rows are identical so any
        # aligned source row works)
        wts_sb = consts.tile([128, SEQ_LEN], f32)

        # ---------------- main loop ----------------
        from functools import partial

        def chunk_epilogue(b, c, embT_t, att_sb, ctx_acc, dn_parts):
            sc_ps = ps_misc.tile([128, CS], f32, tag="m")
            for ab in range(AB):
                nc.tensor.matmul(
                    sc_ps,
                    v_sb[:, ab, :],
                    att_sb[:, ab, :],
                    start=(ab == 0),
                    stop=(ab == AB - 1),
                )
            # all 128 psum rows are identical scores; exp gives the
            # partition-broadcast weights directly + denom partial
            w_bc = wbcp.tile([128, CS], f32)
            nc.scalar.activation(
                out=w_bc, in_=sc_ps,
                func=mybir.ActivationFunctionType.Exp,
                accum_out=dn_parts[:, c : c + 1],
            )
            wslice = wts_sb[32 * b : 32 * b + 1, c * CS : (c + 1) * CS]
            nc.gpsimd.tensor_copy(out=wslice, in_=w_bc[32 * b : 32 * b + 1, :])

            if STAGE >= 2:
                # weighted emb partials: mul on DVE/gpsimd, one wide
                # X-axis reduce per chunk accumulates into ctx_acc col c
                scratch8 = scrp.tile([128, EB, CS], f32)
                gps_n = 3 if (c % 2 == 0) else 4
                for eb in range(EB):
                    eng = nc.gpsimd if eb >= EB - gps_n else nc.vector
                    eng.tensor_mul(
                        scratch8[:, eb, :],
                        embT_t[:, eb, :].bitcast(f32),
                        w_bc,
                    )
                nc.vector.tensor_reduce(
                    out=ctx_acc[:, :, c],
                    in_=scratch8,
                    axis=mybir.AxisListType.X,
                    op=mybir.AluOpType.add,
                )

        def b_tail(b, ctx_acc, dn_parts):
            # normalize weights + context, write out
            brow = wts_sb[32 * b : 32 * b + 1, :]
            if STAGE < 3:
                nc.sync.dma_start(out=wts_out[b : b + 1, :], in_=brow)
                return
            rd = ctxp.tile([128, 1], f32)
            nc.vector.tensor_reduce(
                out=rd, in_=dn_parts,
                axis=mybir.AxisListType.X, op=mybir.AluOpType.add,
            )
            nc.vector.reciprocal(out=rd, in_=rd)
            nc.vector.tensor_scalar_mul(brow, brow, rd[32 * b : 32 * b + 1, 0:1])
            nc.sync.dma_start(out=wts_out[b : b + 1, :], in_=brow)

            ctxv = ctxp.tile([128, EB], f32)
            nc.vector.tensor_reduce(
                out=ctxv, in_=ctx_acc,
                axis=mybir.AxisListType.X, op=mybir.AluOpType.add,
            )
            nc.vector.tensor_scalar_mul(ctxv, ctxv, rd)

            ctxT_ps = ps_misc.tile([EB, 128], f32, tag="m")
            nc.tensor.transpose(ctxT_ps, ctxv, ident)
            ctxT_sb = ctxp.tile([EB, 128], f32)
            nc.scalar.activation(
                out=ctxT_sb, in_=ctxT_ps, func=mybir.ActivationFunctionType.Copy
            )
            nc.sync.dma_start(
                out=ctx_out[b].rearrange("(p f) -> p f", p=EB), in_=ctxT_sb
            )

        # Each chunk's epilogue (scores matmul onward) is emitted AFTER the
        # next chunk's main matmuls so the tensor engine never stalls
        # waiting for tanh of the current chunk.
        pending = []
        for b in range(B_LOC):
            ctx_acc = ctxp.tile([128, EB, NCH], f32)
            dn_parts = ctxp.tile([128, NCH], f32)
            for c in range(NCH):
                embT_t = embp.tile([128, EB, CS], mm_dt)
                nc.sync.dma_start(
                    out=embT_t,
                    in_=embT[b].rearrange("(eb p) s -> p eb s", p=128)[
                        :, :, c * CS : (c + 1) * CS
                    ].bitcast(mm_dt),
                )

                att_sb = attp.tile([128, AB, CS], mm_dt)
                for ab in range(AB):
                    att_ps = ps_att.tile([128, CS], f32)
                    for eb in range(EB):
                        nc.tensor.matmul(
                            att_ps,
                            W_emb_sb[:, eb, ab * 128 : (ab + 1) * 128],
                            embT_t[:, eb, :],
                            start=(eb == 0),
                            stop=(eb == EB - 1),
                        )
                    nc.scalar.activation(
                        out=att_sb[:, ab, :], in_=att_ps,
                        func=mybir.ActivationFunctionType.Tanh,
                        bias=bias_sb[:, ab, b : b + 1],
                    )

                for fn in pending:
                    fn()
                pending = [
                    partial(chunk_epilogue, b, c, embT_t, att_sb, ctx_acc, dn_parts)
                ]
                if c == NCH - 1:
                    pending.append(partial(b_tail, b, ctx_acc, dn_parts))
        for fn in pending:
            fn()

    nc.compile()
    return nc


def _get_nc():
    if "nc" not in _CACHE:
        _CACHE["nc"] = _build(None)
    return _CACHE["nc"]


def _ensure_ntff_hook():
    """Register the axon NTFF profiling hook if the image's antenv lacks it.

    Only used when BASS_KERNEL_TRACE=1 (dev/profiling runs)."""
    import types

    try:
        from antenv.axon_hooks import get_axon_ntff_profile_hook  # noqa: F401
        return
    except ImportError:
        pass
    mod = types.ModuleType("antenv.axon_hooks")
    _h = [None]
    mod.set_axon_ntff_profile_hook = lambda h: _h.__setitem__(0, h)
    mod.get_axon_ntff_profile_hook = lambda: _h[0]
    sys.modules["antenv.axon_hooks"] = mod
    try:
        import antenv

        antenv.axon_hooks = mod
    except ImportError:
        pass
    try:
        from trn_agent_boot.trn_boot import _ntff_profile_via_ctypes

        mod.set_axon_ntff_profile_hook(
            _ntff_profile_via_ctypes("/opt/axon/libaxon_pjrt.so")
        )
    except Exception as e:  # profiling degrades; run still works
        print(f"ntff hook setup failed: {e}", file=sys.stderr)
    # artifact upload has no destination in this container; keep local
    import concourse.bass_utils as bu

    bu.upload_artifacts = lambda tmpdir: f"local://{tmpdir}"


def kernel(hid, emb, W_hid, b_hid, W_emb, b_emb, att_v):
    from concourse.bass_utils import run_bass_kernel_spmd

    nc = _get_nc()

    hid = np.ascontiguousarray(np.asarray(hid, dtype=np.float32))
    emb = np.asarray(emb, dtype=np.float32)
    W_hid = np.ascontiguousarray(np.asarray(W_hid, dtype=np.float32))
    b_hid = np.ascontiguousarray(np.asarray(b_hid, dtype=np.float32))
    W_emb = np.ascontiguousarray(np.asarray(W_emb, dtype=np.float32))
    b_emb = np.ascontiguousarray(np.asarray(b_emb, dtype=np.float32))
    att_v = np.ascontiguousarray(np.asarray(att_v, dtype=np.float32))

    # (S, B, E) -> (B, E, S), contiguous; per-core shards are then views.
    embT_full = np.ascontiguousarray(emb.transpose(1, 2, 0))

    in_maps = []
    for i in range(N_CORES):
        in_maps.append(
            {
                "embT": embT_full[i * B_LOC : (i + 1) * B_LOC],
                "hid": hid[i * B_LOC : (i + 1) * B_LOC],
                "W_emb": W_emb,
                "W_hid": W_hid,
                "b_emb": b_emb,
                "b_hid": b_hid,
                "att_v": att_v,
            }
        )

    trace = os.environ.get("BASS_KERNEL_TRACE", "0") == "1"
    if trace:
        _ensure_ntff_hook()
        tmpdir = os.environ.get("BASS_KERNEL_TRACE_DIR")
        try:
            res = run_bass_kernel_spmd(
                nc, in_maps, core_ids=list(range(N_CORES)), trace=True,
                tmpdir=tmpdir,
            )
        except Exception as e:
            print(f"traced run failed ({e}); retrying without trace", file=sys.stderr)
            res = run_bass_kernel_spmd(nc, in_maps, core_ids=list(range(N_CORES)))
    else:
        res = run_bass_kernel_spmd(nc, in_maps, core_ids=list(range(N_CORES)))
    _CACHE["last_result"] = res

    context = np.concatenate([res.results[i]["ctx"] for i in range(N_CORES)], axis=0)
    weights = np.concatenate([res.results[i]["wts"] for i in range(N_CORES)], axis=0)
    return context, weights


# revision 18
# speedup vs baseline: 1.1797x; 1.1797x over previous
"""Additive (Bahdanau-style) attention kernel for Trainium2, 8 NeuronCores.

Reference computation (per problem):
    hid_att = hid @ W_hid + b_hid                        # (B, A)
    emb_att = einsum('sbe,ea->sba', emb, W_emb) + b_emb  # (S, B, A)
    att     = tanh(emb_att + hid_att[None])              # (S, B, A)
    scores  = einsum('sba,a->bs', att, att_v)            # (B, S)
    weights = softmax(scores, axis=-1)                   # (B, S)
    context = einsum('bs,sbe->be', weights, emb)         # (B, E)
    returns (context, weights)

Sharding: data-parallel over batch. Each of the 8 cores gets 4 batch rows;
projection weights and att_v are replicated. emb is shipped per-core in
(b, e, s) layout so that the E-contraction matmul needs no on-chip
transposes (E lands on the SBUF partition axis naturally), and the
S-contraction (context accumulation) runs on the vector engine as
per-partition dot products over the free (s) axis.

Device dataflow per core (B_LOC=4 local batches, S=2048, E=1024, A=512):
  for b in 4, for chunk c in 4 (512 s each):
    DMA embT[b, :, c] -> SBUF (128 part = e_in, 8 eblk, 512 s)
    PE:  att_ps[ab] (128 a, 512 s) = sum_eb W_emb[eb,ab].T @ embT[eb]   (f32r)
    ACT: att_sb = tanh(att_ps + (hid_att[a,b] + b_emb[a] + b_hid[a]))
    PE:  scores_ps (4, 512 s) = sum_ab v[ab].T(replicated) @ att_sb[ab]
    ACT: w = exp(scores) -> wts_sb row 0 slice   (scores bounded ~|61| < 88)
    GPS: partition_broadcast w -> (128, 512)
    DVE: ctx_acc[b][:, eb, c] = sum_s embT[eb] * w   (tensor_tensor_reduce)
  per-b tail: denom = sum(w row), weights = w/denom, ctx = ctx_acc/denom,
  PE-transpose ctx (128,8)->(8,128), DMA out.
"""

import os
import sys

import numpy as np

if "/opt/trn_rl_repo" not in sys.path:
    sys.path.insert(0, "/opt/trn_rl_repo")

ATT_DIM, HID_DIM, EMB_DIM = 512, 1024, 1024
SEQ_LEN, BATCH = 2048, 32
N_CORES = 8
B_LOC = BATCH // N_CORES          # 4 batch rows per core
EB = EMB_DIM // 128               # 8 e-blocks
AB = ATT_DIM // 128               # 4 a-blocks
CS = 512                          # s chunk size
NCH = SEQ_LEN // CS               # 4 chunks per batch row

# Tensor-engine matmul dtype: "f32r" (relaxed fp32, 4x faster) or "f32".
MM_DT = os.environ.get("KERNEL_MM_DT", "f32r")
# Debug: 1=matmul+tanh+scores+exp+wts, 2=+pass2/ctx, 3=full (normalize)
STAGE = int(os.environ.get("KERNEL_STAGE", "3"))
# eblk index below which pass-2 muls go to DVE (rest to gpsimd)
GPS_SPLIT = int(os.environ.get("KERNEL_GPS_SPLIT", "6"))
# pass-2 implementation: "ttr" (fused mul+reduce, qr.py-style dummy out)
# or "split" (tensor_mul on DVE/gpsimd + wide DVE reduce)
PASS2 = os.environ.get("KERNEL_PASS2", "split")
EMB_BUFS = int(os.environ.get("KERNEL_EMB_BUFS", "5"))

_CACHE = {}


def _build(nc_mod):
    import concourse.bass as bass
    import concourse.tile as tile
    from concourse import bacc, mybir
    from concourse.masks import make_identity

    f32 = mybir.dt.float32
    mm_dt = mybir.dt.float32r if MM_DT == "f32r" else mybir.dt.float32

    nc = bacc.Bacc("TRN2", target_bir_lowering=False, debug=False)

    embT = nc.dram_tensor("embT", [B_LOC, EMB_DIM, SEQ_LEN], f32, kind="ExternalInput").ap()
    hid = nc.dram_tensor("hid", [B_LOC, HID_DIM], f32, kind="ExternalInput").ap()
    W_emb = nc.dram_tensor("W_emb", [EMB_DIM, ATT_DIM], f32, kind="ExternalInput").ap()
    W_hid = nc.dram_tensor("W_hid", [HID_DIM, ATT_DIM], f32, kind="ExternalInput").ap()
    b_emb = nc.dram_tensor("b_emb", [ATT_DIM], f32, kind="ExternalInput").ap()
    b_hid = nc.dram_tensor("b_hid", [ATT_DIM], f32, kind="ExternalInput").ap()
    att_v = nc.dram_tensor("att_v", [ATT_DIM], f32, kind="ExternalInput").ap()
    ctx_out = nc.dram_tensor("ctx", [B_LOC, 128, EB], f32, kind="ExternalOutput").ap()
    wts_out = nc.dram_tensor("wts", [B_LOC, SEQ_LEN], f32, kind="ExternalOutput").ap()

    from contextlib import ExitStack

    with tile.TileContext(nc) as tc, ExitStack() as ctx:
        consts = ctx.enter_context(tc.tile_pool(name="consts", bufs=1))
        embp = ctx.enter_context(tc.tile_pool(name="embp", bufs=EMB_BUFS))
        attp = ctx.enter_context(tc.tile_pool(name="attp", bufs=3))
        wbcp = ctx.enter_context(tc.tile_pool(name="wbcp", bufs=2))
        scrp = ctx.enter_context(tc.tile_pool(name="scrp", bufs=2))
        ctxp = ctx.enter_context(tc.tile_pool(name="ctxp", bufs=2))
        ps_att = ctx.enter_context(tc.tile_pool(name="ps_att", bufs=6, space="PSUM"))
        ps_misc = ctx.enter_context(tc.tile_pool(name="ps_misc", bufs=2, space="PSUM"))

        # ---------------- constants / init ----------------
        ident = consts.tile([128, 128], f32)
        make_identity(nc, ident)

        W_emb_sb = consts.tile([128, EB, ATT_DIM], mm_dt)
        nc.sync.dma_start(
            out=W_emb_sb,
            in_=W_emb.rearrange("(eb p) a -> p eb a", p=128).bitcast(mm_dt),
        )
        # W_hid is only needed during init: borrow an embT pool slot
        W_hid_sb = embp.tile([128, EB, ATT_DIM], f32, tag="embT_t")
        nc.sync.dma_start(
            out=W_hid_sb, in_=W_hid.rearrange("(eb p) a -> p eb a", p=128)
        )

        hid_sb = consts.tile([B_LOC, HID_DIM], f32)
        nc.sync.dma_start(out=hid_sb, in_=hid)

        bh_sb = consts.tile([128, AB], f32)
        nc.sync.dma_start(out=bh_sb, in_=b_hid.rearrange("(ab p) -> p ab", p=128))
        be_sb = consts.tile([128, AB], f32)
        nc.sync.dma_start(out=be_sb, in_=b_emb.rearrange("(ab p) -> p ab", p=128))
        bsum_sb = consts.tile([128, AB], f32)
        nc.vector.tensor_add(bsum_sb, bh_sb, be_sb)

        # v replicated to 128 stationary cols so the scores matmul emits all
        # 128 psum rows = scores (a free partition-broadcast for pass 2).
        # Replication via K=1 matmul: out[m,n] = v_row[0,m] * ones[0,n].
        v_row = consts.tile([1, ATT_DIM], f32)
        nc.sync.dma_start(out=v_row, in_=att_v.unsqueeze(0))
        ones_row = consts.tile([1, 128], f32)
        nc.vector.memset(ones_row, 1.0)
        v_sb = consts.tile([128, AB, 128], mm_dt)
        for ab in range(AB):
            v_ps = ps_misc.tile([128, 128], f32, tag="m")
            nc.tensor.matmul(
                v_ps, v_row[0:1, ab * 128 : (ab + 1) * 128], ones_row,
                start=True, stop=True,
            )
            nc.scalar.activation(
                out=v_sb[:, ab, :], in_=v_ps,
                func=mybir.ActivationFunctionType.Copy,
            )

        # hidT: (B_LOC, E) -> (128 e_in, EB, B_LOC) via PE transposes
        hidT_sb = consts.tile([128, EB, B_LOC], f32)
        for eb in range(EB):
            tp_ps = ps_misc.tile([128, B_LOC], f32, tag="m")
            nc.tensor.transpose(
                tp_ps, hid_sb[:, eb * 128 : (eb + 1) * 128], ident[0:B_LOC, 0:B_LOC]
            )
            nc.scalar.activation(
                out=hidT_sb[:, eb, :], in_=tp_ps,
                func=mybir.ActivationFunctionType.Copy,
            )

        # bias_sb[p, ab, b] = hid_att[b, ab*128+p] + b_emb[.] + b_hid[.]
        bias_sb = consts.tile([128, AB, B_LOC], f32)
        for ab in range(AB):
            hat_ps = ps_misc.tile([128, B_LOC], f32, tag="m")
            for eb in range(EB):
                nc.tensor.matmul(
                    hat_ps,
                    W_hid_sb[:, eb, ab * 128 : (ab + 1) * 128],
                    hidT_sb[:, eb, :],
                    start=(eb == 0),
                    stop=(eb == EB - 1),
                )
            nc.scalar.activation(
                out=bias_sb[:, ab, :], in_=hat_ps,
                func=mybir.ActivationFunctionType.Identity,
                bias=bsum_sb[:, ab : ab + 1],
            )

        # weights staging: row b on partition 32*b (engine ops need
        # 32-aligned start partitions; w_bc rows are identical so any
        # aligned source row works)
        wts_sb = consts.tile([128, SEQ_LEN], f32)

        # ---------------- main loop ----------------
        from functools import partial

        def chunk_epilogue(b, c, embT_t, att_sb, ctx_acc, dn_parts):
            sc_ps = ps_misc.tile([128, CS], f32, tag="m")
            for ab in range(AB):
                nc.tensor.matmul(
                    sc_ps,
                    v_sb[:, ab, :],
                    att_sb[:, ab, :],
                    start=(ab == 0),
                    stop=(ab == AB - 1),
                )
            # all 128 psum rows are identical scores; exp gives the
            # partition-broadcast weights directly + denom partial
            w_bc = wbcp.tile([128, CS], f32)
            nc.scalar.activation(
                out=w_bc, in_=sc_ps,
                func=mybir.ActivationFunctionType.Exp,
                accum_out=dn_parts[:, c : c + 1],
            )
            wslice = wts_sb[32 * b : 32 * b + 1, c * CS : (c + 1) * CS]
            nc.scalar.activation(
                out=wslice, in_=w_bc[32 * b : 32 * b + 1, :],
                func=mybir.ActivationFunctionType.Copy,
            )

            if STAGE >= 2:
                # weighted emb partials: mul on DVE/gpsimd, one wide
                # X-axis reduce per chunk accumulates into ctx_acc col c
                scratch8 = scrp.tile([128, EB, CS], f32)
                for eb in range(EB):
                    eng = nc.gpsimd if eb >= GPS_SPLIT else nc.vector
                    eng.tensor_mul(
                        scratch8[:, eb, :],
                        embT_t[:, eb, :].bitcast(f32),
                        w_bc,
                    )
                nc.vector.tensor_reduce(
                    out=ctx_acc[:, :, c],
                    in_=scratch8,
                    axis=mybir.AxisListType.X,
                    op=mybir.AluOpType.add,
                )

        def b_tail(b, ctx_acc, dn_parts):
            # normalize weights + context, write out
            brow = wts_sb[32 * b : 32 * b + 1, :]
            if STAGE < 3:
                nc.sync.dma_start(out=wts_out[b : b + 1, :], in_=brow)
                return
            rd = ctxp.tile([128, 1], f32)
            nc.vector.tensor_reduce(
                out=rd, in_=dn_parts,
                axis=mybir.AxisListType.X, op=mybir.AluOpType.add,
            )
            nc.vector.reciprocal(out=rd, in_=rd)
            nc.vector.tensor_scalar_mul(brow, brow, rd[32 * b : 32 * b + 1, 0:1])
            nc.sync.dma_start(out=wts_out[b : b + 1, :], in_=brow)

            ctxv = ctxp.tile([128, EB], f32)
            nc.vector.tensor_reduce(
                out=ctxv, in_=ctx_acc,
                axis=mybir.AxisListType.X, op=mybir.AluOpType.add,
            )
            nc.vector.tensor_scalar_mul(ctxv, ctxv, rd)
            # natural (e_in, eb) layout out; host reassembles to (b, e)
            nc.sync.dma_start(out=ctx_out[b], in_=ctxv)

        # Each chunk's epilogue (scores matmul onward) is emitted AFTER the
        # next chunk's main matmuls so the tensor engine never stalls
        # waiting for tanh of the current chunk.
        pending = []
        for b in range(B_LOC):
            ctx_acc = ctxp.tile([128, EB, NCH], f32)
            dn_parts = ctxp.tile([128, NCH], f32)
            for c in range(NCH):
                embT_t = embp.tile([128, EB, CS], mm_dt)
                nc.sync.dma_start(
                    out=embT_t,
                    in_=embT[b].rearrange("(eb p) s -> p eb s", p=128)[
                        :, :, c * CS : (c + 1) * CS
                    ].bitcast(mm_dt),
                )

                att_sb = attp.tile([128, AB, CS], mm_dt)
                for ab in range(AB):
                    att_ps = ps_att.tile([128, CS], f32)
                    for eb in range(EB):
                        nc.tensor.matmul(
                            att_ps,
                            W_emb_sb[:, eb, ab * 128 : (ab + 1) * 128],
                            embT_t[:, eb, :],
                            start=(eb == 0),
                            stop=(eb == EB - 1),
                        )
                    nc.scalar.activation(
                        out=att_sb[:, ab, :], in_=att_ps,
                        func=mybir.ActivationFunctionType.Tanh,
                        bias=bias_sb[:, ab, b : b + 1],
                    )

                for fn in pending:
                    fn()
                pending = [
                    partial(chunk_epilogue, b, c, embT_t, att_sb, ctx_acc, dn_parts)
                ]
                if c == NCH - 1:
                    pending.append(partial(b_tail, b, ctx_acc, dn_parts))
        for fn in pending:
            fn()

    nc.compile()
    return nc


def _get_nc():
    if "nc" not in _CACHE:
        _CACHE["nc"] = _build(None)
    return _CACHE["nc"]


def _ensure_ntff_hook():
    """Register the axon NTFF profiling hook if the image's antenv lacks it.

    Only used when BASS_KERNEL_TRACE=1 (dev/profiling runs)."""
    import types

    try:
        from antenv.axon_hooks import get_axon_ntff_profile_hook  # noqa: F401
        return
    except ImportError:
        pass
    mod = types.ModuleType("antenv.axon_hooks")
    _h = [None]
    mod.set_axon_ntff_profile_hook = lambda h: _h.__setitem__(0, h)
    mod.get_axon_ntff_profile_hook = lambda: _h[0]
    sys.modules["antenv.axon_hooks"] = mod
    try:
        import antenv

        antenv.axon_hooks = mod
    except ImportError:
        pass
    try:
        from trn_agent_boot.trn_boot import _ntff_profile_via_ctypes

        mod.set_axon_ntff_profile_hook(
            _ntff_profile_via_ctypes("/opt/axon/libaxon_pjrt.so")
        )
    except Exception as e:  # profiling degrades; run still works
        print(f"ntff hook setup failed: {e}", file=sys.stderr)
    # artifact upload has no destination in this container; keep local
    import concourse.bass_utils as bu

    bu.upload_artifacts = lambda tmpdir: f"local://{tmpdir}"


def kernel(hid, emb, W_hid, b_hid, W_emb, b_emb, att_v):
    from concourse.bass_utils import run_bass_kernel_spmd

    nc = _get_nc()

    hid = np.ascontiguousarray(np.asarray(hid, dtype=np.float32))
    emb = np.asarray(emb, dtype=np.float32)
    W_hid = np.ascontiguousarray(np.asarray(W_hid, dtype=np.float32))
    b_hid = np.ascontiguousarray(np.asarray(b_hid, dtype=np.float32))
    W_emb = np.ascontiguousarray(np.asarray(W_emb, dtype=np.float32))
    b_emb = np.ascontiguousarray(np.asarray(b_emb, dtype=np.float32))
    att_v = np.ascontiguousarray(np.asarray(att_v, dtype=np.float32))

    # (S, B, E) -> (B, E, S), contiguous; per-core shards are then views.
    embT_full = np.ascontiguousarray(emb.transpose(1, 2, 0))

    in_maps = []
    for i in range(N_CORES):
        in_maps.append(
            {
                "embT": embT_full[i * B_LOC : (i + 1) * B_LOC],
                "hid": hid[i * B_LOC : (i + 1) * B_LOC],
                "W_emb": W_emb,
                "W_hid": W_hid,
                "b_emb": b_emb,
                "b_hid": b_hid,
                "att_v": att_v,
            }
        )

    trace = os.environ.get("BASS_KERNEL_TRACE", "0") == "1"
    if trace:
        _ensure_ntff_hook()
        tmpdir = os.environ.get("BASS_KERNEL_TRACE_DIR")
        try:
            res = run_bass_kernel_spmd(
                nc, in_maps, core_ids=list(range(N_CORES)), trace=True,
                tmpdir=tmpdir,
            )
        except Exception as e:
            print(f"traced run failed ({e}); retrying without trace", file=sys.stderr)
            res = run_bass_kernel_spmd(nc, in_maps, core_ids=list(range(N_CORES)))
    else:
        res = run_bass_kernel_spmd(nc, in_maps, core_ids=list(range(N_CORES)))
    _CACHE["last_result"] = res

    context = np.concatenate(
        [
            res.results[i]["ctx"].transpose(0, 2, 1).reshape(B_LOC, EMB_DIM)
            for i in range(N_CORES)
        ],
        axis=0,
    )
    weights = np.concatenate([res.results[i]["wts"] for i in range(N_CORES)], axis=0)
    return context, weights


# revision 19
# speedup vs baseline: 1.2441x; 1.0546x over previous
"""Additive (Bahdanau-style) attention kernel for Trainium2, 8 NeuronCores.

Reference computation (per problem):
    hid_att = hid @ W_hid + b_hid                        # (B, A)
    emb_att = einsum('sbe,ea->sba', emb, W_emb) + b_emb  # (S, B, A)
    att     = tanh(emb_att + hid_att[None])              # (S, B, A)
    scores  = einsum('sba,a->bs', att, att_v)            # (B, S)
    weights = softmax(scores, axis=-1)                   # (B, S)
    context = einsum('bs,sbe->be', weights, emb)         # (B, E)
    returns (context, weights)

Sharding: data-parallel over batch. Each of the 8 cores gets 4 batch rows;
projection weights and att_v are replicated. emb is shipped per-core in
(b, e, s) layout so that the E-contraction matmul needs no on-chip
transposes (E lands on the SBUF partition axis naturally), and the
S-contraction (context accumulation) runs on the vector engine as
per-partition dot products over the free (s) axis.

Device dataflow per core (B_LOC=4 local batches, S=2048, E=1024, A=512):
  for b in 4, for chunk c in 4 (512 s each):
    DMA embT[b, :, c] -> SBUF (128 part = e_in, 8 eblk, 512 s)
    PE:  att_ps[ab] (128 a, 512 s) = sum_eb W_emb[eb,ab].T @ embT[eb]   (f32r)
    ACT: att_sb = tanh(att_ps + (hid_att[a,b] + b_emb[a] + b_hid[a]))
    PE:  scores_ps (4, 512 s) = sum_ab v[ab].T(replicated) @ att_sb[ab]
    ACT: w = exp(scores) -> wts_sb row 0 slice   (scores bounded ~|61| < 88)
    GPS: partition_broadcast w -> (128, 512)
    DVE: ctx_acc[b][:, eb, c] = sum_s embT[eb] * w   (tensor_tensor_reduce)
  per-b tail: denom = sum(w row), weights = w/denom, ctx = ctx_acc/denom,
  PE-transpose ctx (128,8)->(8,128), DMA out.
"""

import os
import sys

import numpy as np

if "/opt/trn_rl_repo" not in sys.path:
    sys.path.insert(0, "/opt/trn_rl_repo")

ATT_DIM, HID_DIM, EMB_DIM = 512, 1024, 1024
SEQ_LEN, BATCH = 2048, 32
N_CORES = 8
B_LOC = BATCH // N_CORES          # 4 batch rows per core
EB = EMB_DIM // 128               # 8 e-blocks
AB = ATT_DIM // 128               # 4 a-blocks
CS = 512                          # s chunk size
NCH = SEQ_LEN // CS               # 4 chunks per batch row

# Tensor-engine matmul dtype: "f32r" (relaxed fp32, 4x faster) or "f32".
MM_DT = os.environ.get("KERNEL_MM_DT", "f32r")
# Debug: 1=matmul+tanh+scores+exp+wts, 2=+pass2/ctx, 3=full (normalize)
STAGE = int(os.environ.get("KERNEL_STAGE", "3"))
# eblk index below which pass-2 muls go to DVE (rest to gpsimd)
GPS_SPLIT = int(os.environ.get("KERNEL_GPS_SPLIT", "5"))
# eblk index below which the s-reduction runs on DVE (rest on ACT accum)
ACT_RED = int(os.environ.get("KERNEL_ACT_RED", "6"))
# pass-2 implementation: "ttr" (fused mul+reduce, qr.py-style dummy out)
# or "split" (tensor_mul on DVE/gpsimd + wide DVE reduce)
PASS2 = os.environ.get("KERNEL_PASS2", "split")
EMB_BUFS = int(os.environ.get("KERNEL_EMB_BUFS", "5"))

_CACHE = {}


def _build(nc_mod):
    import concourse.bass as bass
    import concourse.tile as tile
    from concourse import bacc, mybir
    from concourse.masks import make_identity

    f32 = mybir.dt.float32
    mm_dt = mybir.dt.float32r if MM_DT == "f32r" else mybir.dt.float32

    nc = bacc.Bacc("TRN2", target_bir_lowering=False, debug=False)

    embT = nc.dram_tensor("embT", [B_LOC, EMB_DIM, SEQ_LEN], f32, kind="ExternalInput").ap()
    hid = nc.dram_tensor("hid", [B_LOC, HID_DIM], f32, kind="ExternalInput").ap()
    W_emb = nc.dram_tensor("W_emb", [EMB_DIM, ATT_DIM], f32, kind="ExternalInput").ap()
    W_hid = nc.dram_tensor("W_hid", [HID_DIM, ATT_DIM], f32, kind="ExternalInput").ap()
    b_emb = nc.dram_tensor("b_emb", [ATT_DIM], f32, kind="ExternalInput").ap()
    b_hid = nc.dram_tensor("b_hid", [ATT_DIM], f32, kind="ExternalInput").ap()
    att_v = nc.dram_tensor("att_v", [ATT_DIM], f32, kind="ExternalInput").ap()
    ctx_out = nc.dram_tensor("ctx", [B_LOC, 128, EB], f32, kind="ExternalOutput").ap()
    wts_out = nc.dram_tensor("wts", [B_LOC, SEQ_LEN], f32, kind="ExternalOutput").ap()

    from contextlib import ExitStack

    with tile.TileContext(nc) as tc, ExitStack() as ctx:
        consts = ctx.enter_context(tc.tile_pool(name="consts", bufs=1))
        embp = ctx.enter_context(tc.tile_pool(name="embp", bufs=EMB_BUFS))
        attp = ctx.enter_context(tc.tile_pool(name="attp", bufs=3))
        wbcp = ctx.enter_context(tc.tile_pool(name="wbcp", bufs=2))
        scrp = ctx.enter_context(tc.tile_pool(name="scrp", bufs=2))
        ctxp = ctx.enter_context(tc.tile_pool(name="ctxp", bufs=2))
        ps_att = ctx.enter_context(tc.tile_pool(name="ps_att", bufs=6, space="PSUM"))
        ps_misc = ctx.enter_context(tc.tile_pool(name="ps_misc", bufs=2, space="PSUM"))

        # ---------------- constants / init ----------------
        ident = consts.tile([128, 128], f32)
        make_identity(nc, ident)

        W_emb_sb = consts.tile([128, EB, ATT_DIM], mm_dt)
        nc.sync.dma_start(
            out=W_emb_sb,
            in_=W_emb.rearrange("(eb p) a -> p eb a", p=128).bitcast(mm_dt),
        )
        # W_hid is only needed during init: borrow an embT pool slot
        W_hid_sb = embp.tile([128, EB, ATT_DIM], f32, tag="embT_t")
        nc.sync.dma_start(
            out=W_hid_sb, in_=W_hid.rearrange("(eb p) a -> p eb a", p=128)
        )

        hid_sb = consts.tile([B_LOC, HID_DIM], f32)
        nc.sync.dma_start(out=hid_sb, in_=hid)

        bh_sb = consts.tile([128, AB], f32)
        nc.sync.dma_start(out=bh_sb, in_=b_hid.rearrange("(ab p) -> p ab", p=128))
        be_sb = consts.tile([128, AB], f32)
        nc.sync.dma_start(out=be_sb, in_=b_emb.rearrange("(ab p) -> p ab", p=128))
        bsum_sb = consts.tile([128, AB], f32)
        nc.vector.tensor_add(bsum_sb, bh_sb, be_sb)

        # v replicated to 128 stationary cols so the scores matmul emits all
        # 128 psum rows = scores (a free partition-broadcast for pass 2).
        # Replication via K=1 matmul: out[m,n] = v_row[0,m] * ones[0,n].
        v_row = consts.tile([1, ATT_DIM], f32)
        nc.sync.dma_start(out=v_row, in_=att_v.unsqueeze(0))
        ones_row = consts.tile([1, 128], f32)
        nc.vector.memset(ones_row, 1.0)
        v_sb = consts.tile([128, AB, 128], mm_dt)
        for ab in range(AB):
            v_ps = ps_misc.tile([128, 128], f32, tag="m")
            nc.tensor.matmul(
                v_ps, v_row[0:1, ab * 128 : (ab + 1) * 128], ones_row,
                start=True, stop=True,
            )
            nc.scalar.activation(
                out=v_sb[:, ab, :], in_=v_ps,
                func=mybir.ActivationFunctionType.Copy,
            )

        # hidT: (B_LOC, E) -> (128 e_in, EB, B_LOC) via PE transposes
        hidT_sb = consts.tile([128, EB, B_LOC], f32)
        for eb in range(EB):
            tp_ps = ps_misc.tile([128, B_LOC], f32, tag="m")
            nc.tensor.transpose(
                tp_ps, hid_sb[:, eb * 128 : (eb + 1) * 128], ident[0:B_LOC, 0:B_LOC]
            )
            nc.scalar.activation(
                out=hidT_sb[:, eb, :], in_=tp_ps,
                func=mybir.ActivationFunctionType.Copy,
            )

        # bias_sb[p, ab, b] = hid_att[b, ab*128+p] + b_emb[.] + b_hid[.]
        bias_sb = consts.tile([128, AB, B_LOC], f32)
        for ab in range(AB):
            hat_ps = ps_misc.tile([128, B_LOC], f32, tag="m")
            for eb in range(EB):
                nc.tensor.matmul(
                    hat_ps,
                    W_hid_sb[:, eb, ab * 128 : (ab + 1) * 128],
                    hidT_sb[:, eb, :],
                    start=(eb == 0),
                    stop=(eb == EB - 1),
                )
            nc.scalar.activation(
                out=bias_sb[:, ab, :], in_=hat_ps,
                func=mybir.ActivationFunctionType.Identity,
                bias=bsum_sb[:, ab : ab + 1],
            )

        # weights staging: row b on partition 32*b (engine ops need
        # 32-aligned start partitions; w_bc rows are identical so any
        # aligned source row works)
        wts_sb = consts.tile([128, SEQ_LEN], f32)

        # ---------------- main loop ----------------
        from functools import partial

        def chunk_epilogue(b, c, embT_t, att_sb, ctx_acc, dn_parts):
            sc_ps = ps_misc.tile([128, CS], f32, tag="m")
            for ab in range(AB):
                nc.tensor.matmul(
                    sc_ps,
                    v_sb[:, ab, :],
                    att_sb[:, ab, :],
                    start=(ab == 0),
                    stop=(ab == AB - 1),
                )
            # all 128 psum rows are identical scores; exp gives the
            # partition-broadcast weights directly + denom partial
            w_bc = wbcp.tile([128, CS], f32)
            nc.scalar.activation(
                out=w_bc, in_=sc_ps,
                func=mybir.ActivationFunctionType.Exp,
                accum_out=dn_parts[:, c : c + 1],
            )
            wslice = wts_sb[32 * b : 32 * b + 1, c * CS : (c + 1) * CS]
            nc.scalar.activation(
                out=wslice, in_=w_bc[32 * b : 32 * b + 1, :],
                func=mybir.ActivationFunctionType.Copy,
            )

            if STAGE >= 2:
                # weighted emb partials: muls split DVE/gpsimd; s-reduction
                # split DVE (wide X-reduce) / ACT (copy with accumulator)
                scratch8 = scrp.tile([128, EB, CS], f32)
                for eb in range(EB):
                    eng = nc.gpsimd if eb >= GPS_SPLIT else nc.vector
                    eng.tensor_mul(
                        scratch8[:, eb, :],
                        embT_t[:, eb, :].bitcast(f32),
                        w_bc,
                    )
                nc.vector.tensor_reduce(
                    out=ctx_acc[:, 0:ACT_RED, c],
                    in_=scratch8[:, 0:ACT_RED, :],
                    axis=mybir.AxisListType.X,
                    op=mybir.AluOpType.add,
                )
                for eb in range(ACT_RED, EB):
                    nc.scalar.activation(
                        out=scratch8[:, eb, :], in_=scratch8[:, eb, :],
                        func=mybir.ActivationFunctionType.Copy,
                        accum_out=ctx_acc[:, eb, c : c + 1],
                    )

        def b_tail(b, ctx_acc, dn_parts):
            # normalize weights + context, write out
            brow = wts_sb[32 * b : 32 * b + 1, :]
            if STAGE < 3:
                nc.sync.dma_start(out=wts_out[b : b + 1, :], in_=brow)
                return
            rd = ctxp.tile([128, 1], f32)
            nc.vector.tensor_reduce(
                out=rd, in_=dn_parts,
                axis=mybir.AxisListType.X, op=mybir.AluOpType.add,
            )
            nc.vector.reciprocal(out=rd, in_=rd)
            nc.vector.tensor_scalar_mul(brow, brow, rd[32 * b : 32 * b + 1, 0:1])
            nc.sync.dma_start(out=wts_out[b : b + 1, :], in_=brow)

            ctxv = ctxp.tile([128, EB], f32)
            nc.vector.tensor_reduce(
                out=ctxv, in_=ctx_acc,
                axis=mybir.AxisListType.X, op=mybir.AluOpType.add,
            )
            nc.vector.tensor_scalar_mul(ctxv, ctxv, rd)
            # natural (e_in, eb) layout out; host reassembles to (b, e)
            nc.sync.dma_start(out=ctx_out[b], in_=ctxv)

        # Each chunk's epilogue (scores matmul onward) is emitted AFTER the
        # next chunk's main matmuls so the tensor engine never stalls
        # waiting for tanh of the current chunk.
        pending = []
        for b in range(B_LOC):
            ctx_acc = ctxp.tile([128, EB, NCH], f32)
            dn_parts = ctxp.tile([128, NCH], f32)
            for c in range(NCH):
                embT_t = embp.tile([128, EB, CS], mm_dt)
                nc.sync.dma_start(
                    out=embT_t,
                    in_=embT[b].rearrange("(eb p) s -> p eb s", p=128)[
                        :, :, c * CS : (c + 1) * CS
                    ].bitcast(mm_dt),
                )

                att_sb = attp.tile([128, AB, CS], mm_dt)
                for ab in range(AB):
                    att_ps = ps_att.tile([128, CS], f32)
                    for eb in range(EB):
                        nc.tensor.matmul(
                            att_ps,
                            W_emb_sb[:, eb, ab * 128 : (ab + 1) * 128],
                            embT_t[:, eb, :],
                            start=(eb == 0),
                            stop=(eb == EB - 1),
                        )
                    nc.scalar.activation(
                        out=att_sb[:, ab, :], in_=att_ps,
                        func=mybir.ActivationFunctionType.Tanh,
                        bias=bias_sb[:, ab, b : b + 1],
                    )

                for fn in pending:
                    fn()
                pending = [
                    partial(chunk_epilogue, b, c, embT_t, att_sb, ctx_acc, dn_parts)
                ]
                if c == NCH - 1:
                    pending.append(partial(b_tail, b, ctx_acc, dn_parts))
        for fn in pending:
            fn()

    nc.compile()
    return nc


def _get_nc():
    if "nc" not in _CACHE:
        _CACHE["nc"] = _build(None)
    return _CACHE["nc"]


def _ensure_ntff_hook():
    """Register the axon NTFF profiling hook if the image's antenv lacks it.

    Only used when BASS_KERNEL_TRACE=1 (dev/profiling runs)."""
    import types

    try:
        from antenv.axon_hooks import get_axon_ntff_profile_hook  # noqa: F401
        return
    except ImportError:
        pass
    mod = types.ModuleType("antenv.axon_hooks")
    _h = [None]
    mod.set_axon_ntff_profile_hook = lambda h: _h.__setitem__(0, h)
    mod.get_axon_ntff_profile_hook = lambda: _h[0]
    sys.modules["antenv.axon_hooks"] = mod
    try:
        import antenv

        antenv.axon_hooks = mod
    except ImportError:
        pass
    try:
        from trn_agent_boot.trn_boot import _ntff_profile_via_ctypes

        mod.set_axon_ntff_profile_hook(
            _ntff_profile_via_ctypes("/opt/axon/libaxon_pjrt.so")
        )
    except Exception as e:  # profiling degrades; run still works
        print(f"ntff hook setup failed: {e}", file=sys.stderr)
    # artifact upload has no destination in this container; keep local
    import concourse.bass_utils as bu

    bu.upload_artifacts = lambda tmpdir: f"local://{tmpdir}"


def kernel(hid, emb, W_hid, b_hid, W_emb, b_emb, att_v):
    from concourse.bass_utils import run_bass_kernel_spmd

    nc = _get_nc()

    hid = np.ascontiguousarray(np.asarray(hid, dtype=np.float32))
    emb = np.asarray(emb, dtype=np.float32)
    W_hid = np.ascontiguousarray(np.asarray(W_hid, dtype=np.float32))
    b_hid = np.ascontiguousarray(np.asarray(b_hid, dtype=np.float32))
    W_emb = np.ascontiguousarray(np.asarray(W_emb, dtype=np.float32))
    b_emb = np.ascontiguousarray(np.asarray(b_emb, dtype=np.float32))
    att_v = np.ascontiguousarray(np.asarray(att_v, dtype=np.float32))

    # (S, B, E) -> (B, E, S), contiguous; per-core shards are then views.
    embT_full = np.ascontiguousarray(emb.transpose(1, 2, 0))

    in_maps = []
    for i in range(N_CORES):
        in_maps.append(
            {
                "embT": embT_full[i * B_LOC : (i + 1) * B_LOC],
                "hid": hid[i * B_LOC : (i + 1) * B_LOC],
                "W_emb": W_emb,
                "W_hid": W_hid,
                "b_emb": b_emb,
                "b_hid": b_hid,
                "att_v": att_v,
            }
        )

    trace = os.environ.get("BASS_KERNEL_TRACE", "0") == "1"
    if trace:
        _ensure_ntff_hook()
        tmpdir = os.environ.get("BASS_KERNEL_TRACE_DIR")
        try:
            res = run_bass_kernel_spmd(
                nc, in_maps, core_ids=list(range(N_CORES)), trace=True,
                tmpdir=tmpdir,
            )
        except Exception as e:
            print(f"traced run failed ({e}); retrying without trace", file=sys.stderr)
            res = run_bass_kernel_spmd(nc, in_maps, core_ids=list(range(N_CORES)))
    else:
        res = run_bass_kernel_spmd(nc, in_maps, core_ids=list(range(N_CORES)))
    _CACHE["last_result"] = res

    context = np.concatenate(
        [
            res.results[i]["ctx"].transpose(0, 2, 1).reshape(B_LOC, EMB_DIM)
            for i in range(N_CORES)
        ],
        axis=0,
    )
    weights = np.concatenate([res.results[i]["wts"] for i in range(N_CORES)], axis=0)
    return context, weights
